# revision 12
# baseline (speedup 1.0000x reference)
"""Trainium2 Bass kernel for BaseLayerWithLoRA.

Computes out = x @ W.T + bias + (x @ A.T) @ B.T for
x [2, 4096, 4096], W [4096, 4096], bias [4096], A [16, 4096], B [4096, 16].

Strategy
--------
The LoRA path is a rank-16 update, so it folds into the weights exactly:

    (x @ A.T) @ B.T = x @ (B @ A).T      =>      W' = W + B @ A

The fold (0.5 GFLOP) and the bias add (one pass over the output) run on
the host; the device executes a single dense GEMM out = x @ W'.T.

Sharding: data-parallel over tokens (B*S = 8192 -> 1024 tokens/core on
8 cores). Each core keeps its x.T shard resident in SBUF and streams
W'.T exactly once.

Precision/speed split (the PE streams one column per cycle for bf16 and
fp32r alike; only fp8 with perf_mode=DoubleRow packs 2 MACs/cell/cycle):
for panels 1..7, 26 of the 32 K-tiles run in bf16 and the last 6 run as
3 fp8e4m3 DoubleRow matmuls (K=256 each) into a second PSUM bank, scaled
by exact powers of two (x*2^4, W*2^9) and recombined at eviction with
(psB * 2^-13) + psA. Measured rel err of this hybrid is ~1.5e-2 against
the 2e-2 gate (bf16-only is 2.3e-3 but ~14% slower).

Engine layout (each DMA_DIRECT2D occupies its issuing engine ~0.6 us,
so DMA issuance and PSUM evictions must not share a queue — an eviction
stuck behind a panel of DMA issues stalls the PE on the bank WAR):
  * scalar (HWDGE): W stream only, batched multi-K-tile descriptors
    (plain 2D SBUF write AP + 3D DRAM read AP — a rearranged 3D *write*
    AP breaks Tile's WAR dep tracking);
  * sync (HWDGE): x loads + output stores;
  * vector: all PSUM evictions/combines;
  * tensor: 36 warm-up matmuls on a zeroed scratch tile (spin the HAM
    clock gate up to 2.4 GHz while the first real operands stream in),
    then the GEMM back-to-back.

Loop order: panel 0 runs pure bf16, K-tiles outermost, so each operand
tile lands just-in-time (no full-panel wait at startup, all 8 banks).
Panels 1..7 run the hybrid with PSUM banks outermost in two 4-token-tile
halves (bank j = bf16, bank 4+j = fp8), so each bank's combine + store
hides under the next bank's compute and the final write-back tail is one
half-tile deep.
"""

import os
import sys

for _p in ("/opt/trn_rl_repo", "/opt/pypackages"):
    if _p not in sys.path:
        sys.path.append(_p)

# The kernel executes on the axon-tunneled NeuronCores via PJRT; a
# JAX_PLATFORMS=cpu pin (used by some reference harnesses) would hide them.
_jp = os.environ.get("JAX_PLATFORMS")
if _jp and "axon" not in _jp:
    del os.environ["JAX_PLATFORMS"]

import ml_dtypes
import numpy as np
import concourse.bacc as bacc
import concourse.mybir as mybir
from concourse.tile import TileContext
from concourse.bass_utils import run_bass_kernel_spmd

F32 = mybir.dt.float32
BF16 = mybir.dt.bfloat16
F8 = mybir.dt.float8e4
NP_BF16 = ml_dtypes.bfloat16
NP_F8 = ml_dtypes.float8_e4m3

BATCH, SEQ, D_IN, D_OUT, RANK = 2, 4096, 4096, 4096, 16
N_CORES = 8
TOK = BATCH * SEQ            # 8192 tokens total
TOK_C = TOK // N_CORES       # 1024 tokens per core
P = 128                      # partitions
NI = D_IN // P               # 32 contraction tiles
O_W = 512                    # output-feature panel width (1 PSUM bank of fp32)
NO = D_OUT // O_W            # 8 output panels
NTOK = TOK_C // P            # 8 token tiles per core
QB = 4                       # K-tiles per bf16 DMA batch
NQ = NI // QB                # 8 x-shard batches
N_WARM = 36                  # PE warm-up matmuls (~3.9 us: bridge until
                             # the first real operand tiles have landed)

NB = 26                      # bf16 K-tiles per bank pass (panels 1..7)
KS = NB * P                  # fp8 region starts at in-feature 3328
NF8 = NI - NB                # 6 fp8 K-tiles
NDR = NF8 // 2               # 3 DoubleRow matmuls (K=256 each)
SX, SW = 16.0, 512.0         # fp8 pre-scales (exact powers of two)
SCOMB = 1.0 / (SX * SW)      # eviction combine factor 2^-13

_NC_CACHE = None


def _build_nc():
    """Trace + schedule + compile the per-core Bass module (SPMD: all 8
    cores run this same program on their own shard)."""
    nc = bacc.Bacc(None, target_bir_lowering=False, debug=False)

    xT = nc.dram_tensor("xT", [D_IN, TOK_C], BF16, kind="ExternalInput")
    x8d = nc.dram_tensor("x8", [NF8 * P, TOK_C], F8, kind="ExternalInput")
    WT = nc.dram_tensor("WT", [D_IN, D_OUT], BF16, kind="ExternalInput")
    W8d = nc.dram_tensor("W8", [NF8 * P, D_OUT], F8, kind="ExternalInput")
    out = nc.dram_tensor("out", [TOK_C, D_OUT], F32, kind="ExternalOutput")

    xT_t = xT.rearrange("(t p) n -> t p n", p=P)
    WT_t = WT.rearrange("(t p) n -> t p n", p=P)
    # Partition-major views for batched loads (2D SBUF write, 3D DRAM read).
    WT_b = WT.rearrange("(t p) n -> p t n", p=P)
    x8_b = x8d.rearrange("(t p) n -> p t n", p=P)
    W8_b = W8d.rearrange("(t p) n -> p t n", p=P)

    DR = mybir.MatmulPerfMode.DoubleRow

    with TileContext(nc) as tc:
        with (
            tc.tile_pool(name="spool", bufs=1) as spool,
            tc.tile_pool(name="xpool", bufs=1) as xpool,
            tc.tile_pool(name="x8pool", bufs=1) as x8pool,
            tc.tile_pool(name="wpool", bufs=3 * (NB // QB)) as wpool,
            tc.tile_pool(name="w2pool", bufs=3) as w2pool,
            tc.tile_pool(name="w8pool", bufs=3 * NDR) as w8pool,
            tc.tile_pool(name="opool", bufs=8) as opool,
            tc.tile_pool(name="o8pool", bufs=2) as o8pool,
            tc.tile_pool(name="pspool", bufs=1, space="PSUM") as pspool,
        ):
            # 8 PSUM banks: psa[j] accumulates the bf16 partial for token
            # tile 4g+j, psb[j] the fp8 partial (panel 0 uses all 8 as
            # plain banks for its 8 token tiles).
            psa = [
                pspool.tile([P, O_W], F32, name=f"psa{j}", tag=f"psa{j}")
                for j in range(4)
            ]
            psb = [
                pspool.tile([P, O_W], F32, name=f"psb{j}", tag=f"psb{j}")
                for j in range(4)
            ]
            bank8 = psa + psb  # panel 0: bank8[tk] <- token tile tk

            # PE warm-up: tiny matmuls on a zeroed scratch tile keep the
            # PE busy (HAM un-throttles to 2.4 GHz after ~3.4 us of
            # activity) while the first real operand DMAs are in flight.
            scr = spool.tile([P, P], BF16, name="scr", tag="scr")
            nc.vector.memset(scr[:], 0.0)
            for i in range(N_WARM):
                nc.tensor.matmul(
                    psa[0][:, 0:P], scr[:], scr[:], start=True, stop=True
                )

            # Resident x.T shard, bf16 part: 32 single-tile DMAs on the
            # sync HWDGE queue (the scalar queue carries the W stream).
            # Singles keep the panel-0 just-in-time cadence: per-tile
            # arrival ~1.1 us vs per-K-tile consumption ~1.7 us.
            xts = []
            for q in range(NQ):
                xt = xpool.tile([P, QB * TOK_C], BF16, name=f"xt{q}", tag=f"xt{q}")
                for i in range(QB):
                    nc.sync.dma_start(
                        out=xt[:, i * TOK_C : (i + 1) * TOK_C],
                        in_=xT_t[q * QB + i],
                    )
                xts.append(xt)
            # fp8 x chunks: chunk c holds K-tiles (NB+2c, NB+2c+1) as the
            # two DoubleRow slots.
            x8s = []
            for c in range(NDR):
                x8c = x8pool.tile(
                    [P, 2 * TOK_C], F8, name=f"x8_{c}", tag=f"x8_{c}"
                )
                nc.sync.dma_start(out=x8c[:], in_=x8_b[:, 2 * c : 2 * c + 2, :])
                x8s.append(x8c)

            def x_ap(t, tk):
                q, r = divmod(t, QB)
                base = r * TOK_C + tk * P
                return xts[q][:, base : base + P]

            def x8_ap(c, tk):
                return x8s[c].rearrange("p (i n) -> p i n", i=2)[
                    :, :, tk * P : (tk + 1) * P
                ]

            def load_w_panel(op, n_tiles, split_first):
                osl = slice(op * O_W, (op + 1) * O_W)
                wts = []
                for q in range(n_tiles // QB):
                    wt = wpool.tile(
                        [P, QB * O_W], BF16, name=f"wt_{op}_{q}", tag="wt"
                    )
                    if split_first and q == 0:
                        for i in range(QB):
                            nc.scalar.dma_start(
                                out=wt[:, i * O_W : (i + 1) * O_W],
                                in_=WT_t[i, :, osl],
                            )
                    else:
                        nc.scalar.dma_start(
                            out=wt[:],
                            in_=WT_b[:, q * QB : (q + 1) * QB, osl],
                        )
                    wts.append(wt)
                rem = n_tiles % QB
                if rem:
                    wt = w2pool.tile(
                        [P, rem * O_W], BF16, name=f"wr_{op}", tag="wr"
                    )
                    nc.scalar.dma_start(
                        out=wt[:],
                        in_=WT_b[:, n_tiles - rem : n_tiles, osl],
                    )
                    wts.append(wt)
                return wts

            def w_ap(wts, t):
                q, r = divmod(t, QB)
                return wts[q][:, r * O_W : (r + 1) * O_W]

            def load_w8_panel(op):
                osl = slice(op * O_W, (op + 1) * O_W)
                w8s = []
                for c in range(NDR):
                    w8 = w8pool.tile(
                        [P, 2 * O_W], F8, name=f"w8_{op}_{c}", tag="w8"
                    )
                    nc.scalar.dma_start(
                        out=w8[:], in_=W8_b[:, 2 * c : 2 * c + 2, osl]
                    )
                    w8s.append(w8)
                return w8s

            def w8_ap(w8s, c):
                return w8s[c].rearrange("p (i n) -> p i n", i=2)

            def store(op, tk, src, sl):
                nc.sync.dma_start(
                    out=out[
                        tk * P : (tk + 1) * P,
                        op * O_W + sl.start : op * O_W + sl.stop,
                    ],
                    in_=src[:, sl],
                )

            # ─── Panel 0: pure bf16, K-tiles outermost (just-in-time) ───
            wts0 = load_w_panel(0, NI, split_first=True)
            for t in range(NI):
                for tk in range(NTOK):
                    nc.tensor.matmul(
                        bank8[tk][:],
                        x_ap(t, tk),
                        w_ap(wts0, t),
                        start=(t == 0),
                        stop=(t == NI - 1),
                    )
            for tk in range(NTOK):
                ot = opool.tile([P, O_W], F32, name=f"ot_0_{tk}", tag="ot")
                nc.vector.tensor_copy(ot[:], bank8[tk][:])
                store(0, tk, ot, slice(0, O_W))

            # ─── Panels 1..7: hybrid, banks outermost ───
            for op in range(1, NO):
                wts = load_w_panel(op, NB, split_first=False)
                w8s = load_w8_panel(op)
                for g in range(2):
                    for j in range(4):
                        tk = 4 * g + j
                        for t in range(NB):
                            nc.tensor.matmul(
                                psa[j][:],
                                x_ap(t, tk),
                                w_ap(wts, t),
                                start=(t == 0),
                                stop=(t == NB - 1),
                            )
                        for c in range(NDR):
                            nc.tensor.matmul(
                                psb[j][:],
                                x8_ap(c, tk),
                                w8_ap(w8s, c),
                                start=(c == 0),
                                stop=(c == NDR - 1),
                                perf_mode=DR,
                            )
                        # Combine + evict: ot = psb * 2^-13 + psa. The HW
                        # allows only one PSUM input per DVE instruction
                        # (NCC_IBVF027), so stage the scaled fp8 partial
                        # through SBUF first.
                        last = op == NO - 1 and tk == NTOK - 1
                        o8 = o8pool.tile(
                            [P, O_W], F32, name=f"o8_{op}_{tk}", tag="o8"
                        )
                        nc.vector.tensor_scalar_mul(o8[:], psb[j][:], SCOMB)
                        ot = opool.tile(
                            [P, O_W], F32, name=f"ot_{op}_{tk}", tag="ot"
                        )
                        nsp = 2 if last else 1
                        h = O_W // nsp
                        for s in range(nsp):
                            sl = slice(s * h, (s + 1) * h)
                            nc.vector.tensor_add(
                                ot[:, sl], o8[:, sl], psa[j][:, sl]
                            )
                            store(op, tk, ot, sl)

    nc.compile()
    return nc


def _get_nc():
    global _NC_CACHE
    if _NC_CACHE is None:
        _NC_CACHE = _build_nc()
    return _NC_CACHE


def _prep_inputs(x, W, bias, A, B):
    """Host-side layout prep + sharding. Returns per-core input maps."""
    x_flat = np.ascontiguousarray(x, dtype=np.float32).reshape(TOK, D_IN)
    # Fold the rank-16 LoRA update into the weights: W' = W + B @ A.
    Wf = np.asarray(W, dtype=np.float32) + np.asarray(
        B, dtype=np.float32
    ) @ np.asarray(A, dtype=np.float32)
    WfT = np.ascontiguousarray(Wf.T)
    WT = WfT.astype(NP_BF16)
    W8 = np.ascontiguousarray(WfT[KS:] * SW).astype(NP_F8)
    x16 = x_flat.astype(NP_BF16)
    x8f = x_flat[:, KS:] * SX
    in_maps = []
    for c in range(N_CORES):
        rows = slice(c * TOK_C, (c + 1) * TOK_C)
        xT_c = np.ascontiguousarray(x16[rows].T)
        x8_c = np.ascontiguousarray(x8f[rows].T).astype(NP_F8)
        in_maps.append({"xT": xT_c, "x8": x8_c, "WT": WT, "W8": W8})
    return in_maps


def _run(inputs, trace=False, trace_cores=None):
    nc = _get_nc()
    in_maps = _prep_inputs(**inputs)
    res = run_bass_kernel_spmd(
        nc,
        in_maps,
        core_ids=list(range(N_CORES)),
        trace=trace,
        trace_cores=trace_cores,
    )
    bias = np.asarray(inputs["bias"], dtype=np.float32)
    shards = [res.results[c]["out"] + bias for c in range(N_CORES)]
    full = np.concatenate(shards, axis=0).reshape(BATCH, SEQ, D_OUT)
    return full, res


def kernel(**inputs):
    full, _ = _run(inputs, trace=False)
    return full


if __name__ == "__main__":
    rng = np.random.default_rng(0)
    inputs = {
        "x": rng.standard_normal((BATCH, SEQ, D_IN), dtype=np.float32),
        "W": rng.standard_normal((D_OUT, D_IN), dtype=np.float32) * 0.02,
        "bias": rng.standard_normal((D_OUT,), dtype=np.float32) * 0.02,
        "A": rng.standard_normal((RANK, D_IN), dtype=np.float32) * 0.02,
        "B": rng.standard_normal((D_OUT, RANK), dtype=np.float32) * 0.02,
    }
    got = kernel(**inputs)
    x64 = inputs["x"].reshape(TOK, D_IN).astype(np.float64)
    exp = x64 @ inputs["W"].astype(np.float64).T + inputs["bias"]
    exp += (x64 @ inputs["A"].astype(np.float64).T) @ inputs["B"].astype(np.float64).T
    exp = exp.reshape(BATCH, SEQ, D_OUT)
    rel = np.linalg.norm(got - exp) / np.linalg.norm(exp)
    print("self-check relative error:", rel)


# revision 13
# speedup vs baseline: 1.1096x; 1.1096x over previous
"""Trainium2 Bass kernel for BaseLayerWithLoRA.

Computes out = x @ W.T + bias + (x @ A.T) @ B.T for
x [2, 4096, 4096], W [4096, 4096], bias [4096], A [16, 4096], B [4096, 16].

Strategy
--------
The LoRA path is a rank-16 update, so it folds into the weights exactly:

    (x @ A.T) @ B.T = x @ (B @ A).T      =>      W' = W + B @ A

The fold (0.5 GFLOP) and the bias add (one pass over the output) run on
the host; the device executes a single dense GEMM out = x @ W'.T in
bf16 (rel err ~2.3e-3, well under the 2e-2 gate). bf16 streams at the
PE's full 1 column/cycle rate — same as fp32r — while halving all DMA
traffic. (fp8 DoubleRow was tried and measured slower: the doubled
moving operand still streams 1 column/cycle into an fp32 PSUM bank, and
the added power drops the PE from 2.4 to 2.0 GHz chip-wide.)

Sharding: data-parallel over tokens (B*S = 8192 -> 1024 tokens/core on
8 cores). Each core keeps its x.T shard (8 MiB bf16) resident in SBUF
and streams W'.T exactly once (32 MiB bf16). 2048 matmuls of
[128,128]x[128,512] at the measured 216 ns warm issue gap put the PE at
~96% of its streaming roofline.

Engine layout (each DMA_DIRECT2D occupies its issuing engine ~0.6 us,
so DMA issuance and PSUM evictions must not share a queue — an eviction
stuck behind a panel of DMA issues stalls the PE on the bank WAR):
  * scalar (HWDGE): W stream only, batched 4 K-tiles per descriptor
    (plain 2D SBUF write AP + 3D DRAM read AP — a rearranged 3D *write*
    AP breaks Tile's WAR dependency tracking);
  * sync (HWDGE): x loads (singles, just-in-time cadence) + output
    stores;
  * vector: all PSUM evictions;
  * tensor: 36 warm-up matmuls on a zeroed scratch tile (spin the HAM
    clock gate up to 2.4 GHz while the first real operands stream in),
    then 2048 GEMM matmuls back-to-back.

Loop order: panel 0 iterates K-tiles outermost so each operand tile
lands just-in-time (no full-panel wait at startup); panels 1..7 iterate
PSUM banks outermost so each bank's eviction + store hides under the
next bank's 7 us of compute and the final write-back tail is one
half-tile deep.
"""

import os
import sys

for _p in ("/opt/trn_rl_repo", "/opt/pypackages"):
    if _p not in sys.path:
        sys.path.append(_p)

# The kernel executes on the axon-tunneled NeuronCores via PJRT; a
# JAX_PLATFORMS=cpu pin (used by some reference harnesses) would hide them.
_jp = os.environ.get("JAX_PLATFORMS")
if _jp and "axon" not in _jp:
    del os.environ["JAX_PLATFORMS"]

import ml_dtypes
import numpy as np
import concourse.bacc as bacc
import concourse.mybir as mybir
from concourse.tile import TileContext
from concourse.bass_utils import run_bass_kernel_spmd

F32 = mybir.dt.float32
BF16 = mybir.dt.bfloat16
NP_BF16 = ml_dtypes.bfloat16

BATCH, SEQ, D_IN, D_OUT, RANK = 2, 4096, 4096, 4096, 16
N_CORES = 8
TOK = BATCH * SEQ            # 8192 tokens total
TOK_C = TOK // N_CORES       # 1024 tokens per core
P = 128                      # partitions
NI = D_IN // P               # 32 contraction tiles
O_W = 512                    # output-feature panel width (1 PSUM bank of fp32)
NO = D_OUT // O_W            # 8 output panels
NTOK = TOK_C // P            # 8 token tiles per core = 8 PSUM banks
QB = 4                       # K-tiles per W DMA batch
NQ = NI // QB                # 8 batches per panel / per x shard
N_WARM = 36                  # PE warm-up matmuls (~3.9 us: bridge until
                             # the first real operand tiles have landed)

_NC_CACHE = None


def _build_nc():
    """Trace + schedule + compile the per-core Bass module (SPMD: all 8
    cores run this same program on their own shard)."""
    nc = bacc.Bacc(None, target_bir_lowering=False, debug=False)

    xT = nc.dram_tensor("xT", [D_IN, TOK_C], BF16, kind="ExternalInput")
    WT = nc.dram_tensor("WT", [D_IN, D_OUT], BF16, kind="ExternalInput")
    out = nc.dram_tensor("out", [TOK_C, D_OUT], F32, kind="ExternalOutput")

    xT_t = xT.rearrange("(t p) n -> t p n", p=P)
    WT_t = WT.rearrange("(t p) n -> t p n", p=P)
    # Partition-major view so a 4-K-tile batch lands in one DMA with a
    # plain 2D SBUF write AP (a rearranged 3D *write* AP breaks Tile's
    # WAR dependency tracking; a 3D DRAM *read* AP is safe).
    WT_b = WT.rearrange("(t p) n -> p t n", p=P)

    with TileContext(nc) as tc:
        with (
            tc.tile_pool(name="spool", bufs=1) as spool,
            tc.tile_pool(name="xpool", bufs=1) as xpool,
            tc.tile_pool(name="wpool", bufs=2 * NQ) as wpool,
            tc.tile_pool(name="opool", bufs=8) as opool,
            tc.tile_pool(name="pspool", bufs=1, space="PSUM") as pspool,
        ):
            psums = [
                pspool.tile([P, O_W], F32, name=f"ps{tk}", tag=f"ps{tk}")
                for tk in range(NTOK)
            ]

            # PE warm-up: tiny matmuls on a zeroed scratch tile keep the
            # PE busy (HAM un-throttles to 2.4 GHz after ~3.4 us of
            # activity) while the first real operand DMAs are in flight.
            # They write bank 0, which the first real start=True matmul
            # clears anyway.
            scr = spool.tile([P, P], BF16, name="scr", tag="scr")
            nc.vector.memset(scr[:], 0.0)
            for i in range(N_WARM):
                nc.tensor.matmul(
                    psums[0][:, 0:P], scr[:], scr[:], start=True, stop=True
                )

            # Resident x.T shard: 32 single-tile DMAs on the sync HWDGE
            # queue (the scalar queue carries the W stream). Singles keep
            # the panel-0 just-in-time cadence: per-tile arrival ~1.1 us
            # vs per-K-tile consumption ~1.7 us.
            xts = []
            for q in range(NQ):
                xt = xpool.tile([P, QB * TOK_C], BF16, name=f"xt{q}", tag=f"xt{q}")
                for i in range(QB):
                    nc.sync.dma_start(
                        out=xt[:, i * TOK_C : (i + 1) * TOK_C],
                        in_=xT_t[q * QB + i],
                    )
                xts.append(xt)

            def x_ap(t, tk):
                q, r = divmod(t, QB)
                base = r * TOK_C + tk * P
                return xts[q][:, base : base + P]

            def load_w_panel(op, split_first):
                osl = slice(op * O_W, (op + 1) * O_W)
                wts = []
                for q in range(NQ):
                    wt = wpool.tile(
                        [P, QB * O_W], BF16, name=f"wt_{op}_{q}", tag="wt"
                    )
                    if split_first and q == 0:
                        for i in range(QB):
                            nc.scalar.dma_start(
                                out=wt[:, i * O_W : (i + 1) * O_W],
                                in_=WT_t[i, :, osl],
                            )
                    else:
                        nc.scalar.dma_start(
                            out=wt[:],
                            in_=WT_b[:, q * QB : (q + 1) * QB, osl],
                        )
                    wts.append(wt)
                return wts

            def w_ap(wts, t):
                q, r = divmod(t, QB)
                return wts[q][:, r * O_W : (r + 1) * O_W]

            def evict(op, tk, last=False):
                osl = op * O_W
                ot = opool.tile([P, O_W], F32, name=f"ot_{op}_{tk}", tag="ot")
                # Split the final eviction so its store overlaps the
                # second half's copy (shorter end-of-kernel tail).
                nsp = 2 if last else 1
                h = O_W // nsp
                for s in range(nsp):
                    sl = slice(s * h, (s + 1) * h)
                    nc.vector.tensor_copy(ot[:, sl], psums[tk][:, sl])
                    nc.sync.dma_start(
                        out=out[tk * P : (tk + 1) * P, osl + s * h : osl + (s + 1) * h],
                        in_=ot[:, sl],
                    )

            # Panel 0: K-tiles outermost — operand tiles stream in just
            # ahead of their matmuls, PE starts ~8 us in.
            wts0 = load_w_panel(0, split_first=True)
            for t in range(NI):
                for tk in range(NTOK):
                    nc.tensor.matmul(
                        psums[tk][:],
                        x_ap(t, tk),
                        w_ap(wts0, t),
                        start=(t == 0),
                        stop=(t == NI - 1),
                    )
            for tk in range(NTOK):
                evict(0, tk)

            # Panels 1..7: banks outermost — evictions and stores hide
            # under the next bank's compute.
            for op in range(1, NO):
                wts = load_w_panel(op, split_first=False)
                for tk in range(NTOK):
                    for t in range(NI):
                        nc.tensor.matmul(
                            psums[tk][:],
                            x_ap(t, tk),
                            w_ap(wts, t),
                            start=(t == 0),
                            stop=(t == NI - 1),
                        )
                    evict(op, tk, last=(op == NO - 1 and tk == NTOK - 1))

    nc.compile()
    return nc


def _get_nc():
    global _NC_CACHE
    if _NC_CACHE is None:
        _NC_CACHE = _build_nc()
    return _NC_CACHE


def _prep_inputs(x, W, bias, A, B):
    """Host-side layout prep + sharding. Returns per-core input maps."""
    x_flat = np.ascontiguousarray(x, dtype=np.float32).reshape(TOK, D_IN)
    # Fold the rank-16 LoRA update into the weights: W' = W + B @ A.
    Wf = np.asarray(W, dtype=np.float32) + np.asarray(
        B, dtype=np.float32
    ) @ np.asarray(A, dtype=np.float32)
    WT = np.ascontiguousarray(Wf.T).astype(NP_BF16)
    x16 = x_flat.astype(NP_BF16)
    in_maps = []
    for c in range(N_CORES):
        xT_c = np.ascontiguousarray(x16[c * TOK_C : (c + 1) * TOK_C, :].T)
        in_maps.append({"xT": xT_c, "WT": WT})
    return in_maps


def _run(inputs, trace=False, trace_cores=None):
    nc = _get_nc()
    in_maps = _prep_inputs(**inputs)
    res = run_bass_kernel_spmd(
        nc,
        in_maps,
        core_ids=list(range(N_CORES)),
        trace=trace,
        trace_cores=trace_cores,
    )
    bias = np.asarray(inputs["bias"], dtype=np.float32)
    shards = [res.results[c]["out"] + bias for c in range(N_CORES)]
    full = np.concatenate(shards, axis=0).reshape(BATCH, SEQ, D_OUT)
    return full, res


def kernel(**inputs):
    full, _ = _run(inputs, trace=False)
    return full


if __name__ == "__main__":
    rng = np.random.default_rng(0)
    inputs = {
        "x": rng.standard_normal((BATCH, SEQ, D_IN), dtype=np.float32),
        "W": rng.standard_normal((D_OUT, D_IN), dtype=np.float32) * 0.02,
        "bias": rng.standard_normal((D_OUT,), dtype=np.float32) * 0.02,
        "A": rng.standard_normal((RANK, D_IN), dtype=np.float32) * 0.02,
        "B": rng.standard_normal((D_OUT, RANK), dtype=np.float32) * 0.02,
    }
    got = kernel(**inputs)
    x64 = inputs["x"].reshape(TOK, D_IN).astype(np.float64)
    exp = x64 @ inputs["W"].astype(np.float64).T + inputs["bias"]
    exp += (x64 @ inputs["A"].astype(np.float64).T) @ inputs["B"].astype(np.float64).T
    exp = exp.reshape(BATCH, SEQ, D_OUT)
    rel = np.linalg.norm(got - exp) / np.linalg.norm(exp)
    print("self-check relative error:", rel)


# revision 17
# speedup vs baseline: 1.1120x; 1.0022x over previous
"""Trainium2 Bass kernel for BaseLayerWithLoRA.

Computes out = x @ W.T + bias + (x @ A.T) @ B.T for
x [2, 4096, 4096], W [4096, 4096], bias [4096], A [16, 4096], B [4096, 16].

Strategy
--------
The LoRA path is a rank-16 update, so it folds into the weights exactly:

    (x @ A.T) @ B.T = x @ (B @ A).T      =>      W' = W + B @ A

The fold (0.5 GFLOP) and the bias add (one pass over the output) run on
the host; the device executes a single dense GEMM out = x @ W'.T in
bf16 (rel err ~2.3e-3, well under the 2e-2 gate). bf16 streams at the
PE's full 1 column/cycle rate — same as fp32r — while halving all DMA
traffic. (fp8 DoubleRow was tried and measured slower: the doubled
moving operand still streams 1 column/cycle into an fp32 PSUM bank, and
the added power drops the PE from 2.4 to 2.0 GHz chip-wide.)

Sharding: data-parallel over tokens (B*S = 8192 -> 1024 tokens/core on
8 cores). Each core keeps its x.T shard (8 MiB bf16) resident in SBUF
and streams W'.T exactly once (32 MiB bf16). 2048 matmuls of
[128,128]x[128,512] at the measured 216 ns warm issue gap put the PE at
~96% of its streaming roofline.

Engine layout (each DMA_DIRECT2D occupies its issuing engine ~0.6 us,
so DMA issuance and PSUM evictions must not share a queue — an eviction
stuck behind a panel of DMA issues stalls the PE on the bank WAR):
  * scalar (HWDGE): W stream only, batched 4 K-tiles per descriptor
    (plain 2D SBUF write AP + 3D DRAM read AP — a rearranged 3D *write*
    AP breaks Tile's WAR dependency tracking);
  * sync (HWDGE): x loads (singles, just-in-time cadence) + output
    stores;
  * vector: all PSUM evictions;
  * tensor: 36 warm-up matmuls on a zeroed scratch tile (spin the HAM
    clock gate up to 2.4 GHz while the first real operands stream in),
    then 2048 GEMM matmuls back-to-back.

Loop order: panel 0 iterates K-tiles outermost so each operand tile
lands just-in-time (no full-panel wait at startup); panels 1..7 iterate
PSUM banks outermost so each bank's eviction + store hides under the
next bank's 7 us of compute and the final write-back tail is one
half-tile deep.
"""

import os
import sys

for _p in ("/opt/trn_rl_repo", "/opt/pypackages"):
    if _p not in sys.path:
        sys.path.append(_p)

# The kernel executes on the axon-tunneled NeuronCores via PJRT; a
# JAX_PLATFORMS=cpu pin (used by some reference harnesses) would hide them.
_jp = os.environ.get("JAX_PLATFORMS")
if _jp and "axon" not in _jp:
    del os.environ["JAX_PLATFORMS"]

import ml_dtypes
import numpy as np
import concourse.bacc as bacc
import concourse.mybir as mybir
from concourse.tile import TileContext
from concourse.bass_utils import run_bass_kernel_spmd

F32 = mybir.dt.float32
BF16 = mybir.dt.bfloat16
NP_BF16 = ml_dtypes.bfloat16

BATCH, SEQ, D_IN, D_OUT, RANK = 2, 4096, 4096, 4096, 16
N_CORES = 8
TOK = BATCH * SEQ            # 8192 tokens total
TOK_C = TOK // N_CORES       # 1024 tokens per core
P = 128                      # partitions
NI = D_IN // P               # 32 contraction tiles
O_W = 512                    # output-feature panel width (1 PSUM bank of fp32)
NO = D_OUT // O_W            # 8 output panels
NTOK = TOK_C // P            # 8 token tiles per core = 8 PSUM banks
QB = 4                       # K-tiles per W DMA batch
NQ = NI // QB                # 8 batches per panel / per x shard
N_WARM = 28                  # PE warm-up matmuls (~3 us: bridge until
                             # the first real operand tiles have landed)

_NC_CACHE = None


def _build_nc():
    """Trace + schedule + compile the per-core Bass module (SPMD: all 8
    cores run this same program on their own shard)."""
    nc = bacc.Bacc(None, target_bir_lowering=False, debug=False)

    xT = nc.dram_tensor("xT", [D_IN, TOK_C], BF16, kind="ExternalInput")
    WT = nc.dram_tensor("WT", [D_IN, D_OUT], BF16, kind="ExternalInput")
    out = nc.dram_tensor("out", [TOK_C, D_OUT], F32, kind="ExternalOutput")

    xT_t = xT.rearrange("(t p) n -> t p n", p=P)
    WT_t = WT.rearrange("(t p) n -> t p n", p=P)
    # Partition-major view so a 4-K-tile batch lands in one DMA with a
    # plain 2D SBUF write AP (a rearranged 3D *write* AP breaks Tile's
    # WAR dependency tracking; a 3D DRAM *read* AP is safe).
    WT_b = WT.rearrange("(t p) n -> p t n", p=P)

    with TileContext(nc) as tc:
        with (
            tc.tile_pool(name="spool", bufs=1) as spool,
            tc.tile_pool(name="xpool", bufs=1) as xpool,
            tc.tile_pool(name="wpool", bufs=2 * NQ) as wpool,
            tc.tile_pool(name="opool", bufs=8) as opool,
            tc.tile_pool(name="pspool", bufs=1, space="PSUM") as pspool,
        ):
            psums = [
                pspool.tile([P, O_W], F32, name=f"ps{tk}", tag=f"ps{tk}")
                for tk in range(NTOK)
            ]

            # PE warm-up: tiny matmuls on a zeroed scratch tile keep the
            # PE busy (HAM un-throttles to 2.4 GHz after ~3.4 us of
            # activity) while the first real operand DMAs are in flight.
            # They write bank 0, which the first real start=True matmul
            # clears anyway.
            scr = spool.tile([P, P], BF16, name="scr", tag="scr")
            nc.vector.memset(scr[:], 0.0)
            for i in range(N_WARM):
                nc.tensor.matmul(
                    psums[0][:, 0:P], scr[:], scr[:], start=True, stop=True
                )

            # Resident x.T shard: 32 single-tile DMAs on the sync HWDGE
            # queue (the scalar queue carries the W stream). Singles keep
            # the panel-0 just-in-time cadence: per-tile arrival ~1.1 us
            # vs per-K-tile consumption ~1.7 us.
            xts = []
            for q in range(NQ):
                xt = xpool.tile([P, QB * TOK_C], BF16, name=f"xt{q}", tag=f"xt{q}")
                for i in range(QB):
                    nc.sync.dma_start(
                        out=xt[:, i * TOK_C : (i + 1) * TOK_C],
                        in_=xT_t[q * QB + i],
                    )
                xts.append(xt)

            def x_ap(t, tk):
                q, r = divmod(t, QB)
                base = r * TOK_C + tk * P
                return xts[q][:, base : base + P]

            def load_w_panel(op, split_first):
                osl = slice(op * O_W, (op + 1) * O_W)
                wts = []
                for q in range(NQ):
                    wt = wpool.tile(
                        [P, QB * O_W], BF16, name=f"wt_{op}_{q}", tag="wt"
                    )
                    if split_first and q == 0:
                        for i in range(QB):
                            nc.scalar.dma_start(
                                out=wt[:, i * O_W : (i + 1) * O_W],
                                in_=WT_t[i, :, osl],
                            )
                    else:
                        nc.scalar.dma_start(
                            out=wt[:],
                            in_=WT_b[:, q * QB : (q + 1) * QB, osl],
                        )
                    wts.append(wt)
                return wts

            def w_ap(wts, t):
                q, r = divmod(t, QB)
                return wts[q][:, r * O_W : (r + 1) * O_W]

            def evict(op, tk, last=False):
                osl = op * O_W
                ot = opool.tile([P, O_W], F32, name=f"ot_{op}_{tk}", tag="ot")
                # Split the final eviction so its store overlaps the
                # second half's copy (shorter end-of-kernel tail).
                nsp = 2 if last else 1
                h = O_W // nsp
                for s in range(nsp):
                    sl = slice(s * h, (s + 1) * h)
                    nc.vector.tensor_copy(ot[:, sl], psums[tk][:, sl])
                    nc.sync.dma_start(
                        out=out[tk * P : (tk + 1) * P, osl + s * h : osl + (s + 1) * h],
                        in_=ot[:, sl],
                    )

            # Panel 0: K-tiles outermost — operand tiles stream in just
            # ahead of their matmuls, PE starts ~8 us in.
            wts0 = load_w_panel(0, split_first=True)
            for t in range(NI):
                for tk in range(NTOK):
                    nc.tensor.matmul(
                        psums[tk][:],
                        x_ap(t, tk),
                        w_ap(wts0, t),
                        start=(t == 0),
                        stop=(t == NI - 1),
                    )
            for tk in range(NTOK):
                evict(0, tk)

            # Panels 1..7: banks outermost — evictions and stores hide
            # under the next bank's compute.
            for op in range(1, NO):
                wts = load_w_panel(op, split_first=False)
                for tk in range(NTOK):
                    for t in range(NI):
                        nc.tensor.matmul(
                            psums[tk][:],
                            x_ap(t, tk),
                            w_ap(wts, t),
                            start=(t == 0),
                            stop=(t == NI - 1),
                        )
                    evict(op, tk, last=(op == NO - 1 and tk == NTOK - 1))

    nc.compile()
    return nc


def _get_nc():
    global _NC_CACHE
    if _NC_CACHE is None:
        _NC_CACHE = _build_nc()
    return _NC_CACHE


def _prep_inputs(x, W, bias, A, B):
    """Host-side layout prep + sharding. Returns per-core input maps."""
    x_flat = np.ascontiguousarray(x, dtype=np.float32).reshape(TOK, D_IN)
    # Fold the rank-16 LoRA update into the weights: W' = W + B @ A.
    Wf = np.asarray(W, dtype=np.float32) + np.asarray(
        B, dtype=np.float32
    ) @ np.asarray(A, dtype=np.float32)
    WT = np.ascontiguousarray(Wf.T).astype(NP_BF16)
    x16 = x_flat.astype(NP_BF16)
    in_maps = []
    for c in range(N_CORES):
        xT_c = np.ascontiguousarray(x16[c * TOK_C : (c + 1) * TOK_C, :].T)
        in_maps.append({"xT": xT_c, "WT": WT})
    return in_maps


def _run(inputs, trace=False, trace_cores=None):
    nc = _get_nc()
    in_maps = _prep_inputs(**inputs)
    res = run_bass_kernel_spmd(
        nc,
        in_maps,
        core_ids=list(range(N_CORES)),
        trace=trace,
        trace_cores=trace_cores,
    )
    bias = np.asarray(inputs["bias"], dtype=np.float32)
    shards = [res.results[c]["out"] + bias for c in range(N_CORES)]
    full = np.concatenate(shards, axis=0).reshape(BATCH, SEQ, D_OUT)
    return full, res


def kernel(**inputs):
    full, _ = _run(inputs, trace=False)
    return full


if __name__ == "__main__":
    rng = np.random.default_rng(0)
    inputs = {
        "x": rng.standard_normal((BATCH, SEQ, D_IN), dtype=np.float32),
        "W": rng.standard_normal((D_OUT, D_IN), dtype=np.float32) * 0.02,
        "bias": rng.standard_normal((D_OUT,), dtype=np.float32) * 0.02,
        "A": rng.standard_normal((RANK, D_IN), dtype=np.float32) * 0.02,
        "B": rng.standard_normal((D_OUT, RANK), dtype=np.float32) * 0.02,
    }
    got = kernel(**inputs)
    x64 = inputs["x"].reshape(TOK, D_IN).astype(np.float64)
    exp = x64 @ inputs["W"].astype(np.float64).T + inputs["bias"]
    exp += (x64 @ inputs["A"].astype(np.float64).T) @ inputs["B"].astype(np.float64).T
    exp = exp.reshape(BATCH, SEQ, D_OUT)
    rel = np.linalg.norm(got - exp) / np.linalg.norm(exp)
    print("self-check relative error:", rel)
